# revision 24
# baseline (speedup 1.0000x reference)
"""Trainium2 Bass kernel for nn_MultiHeadSelfAttention_88725434400988.

Self-contained: accepts FULL inputs, shards batch B=256 over 8 NeuronCores
(32 per core), runs one SPMD Bass program, gathers the FULL output.

Per-core algorithm (B_CORE=32, S=8, F=32, E=64, A=64, NH=2):
  - Hs, Wq, Wk, Wv, Wres cast to fp16 on host (PE matmuls run 1 cyc/row,
    fp32 PSUM accumulation; end-to-end error vs fp32 reference ~3.6e-3
    absmax / 1.9e-3 l2-relative).
  - All transposed/tiled operands are pre-laid-out on HOST into HBM buffers
    whose per-partition bytes are contiguous, so every big DMA moves
    multi-KB descriptor runs.
  - Working label order for attention rows/cols: p = jh*128 + f*4 + sp,
    where the original index is sp*64 + 2f + jh (jh == psum partition half
    of the projection output; the order makes gather copies 3-dim strided
    with 32B-contiguous runs).
  - QK projection: lhsT = 128-col tiles of W, rhs = Hs^T; psum groups of 4
    tiles (128=(jh,a), 4, 256=(b,s)); one batched copy per (group, jh).
    The jh=1 halves are staged and partition-shifted with one SBUF->SBUF
    DMA (engines cannot cross partitions; DMA can).
  - v: lhsT = host-transposed Hs rows (e, 128 rows) per (b,nh) pair ->
    v_all bf16 (128=sigma tile, bn, 2, 64).
  - Attention is TRANSPOSE-FREE: Z^T = k_chunk.T @ q (sigma on partition),
    exp on ScalarE -> bf16 (fp32 range, no overflow at |z|<=49; fp16 would
    overflow), denominators via ones-vector matmul (free), reciprocal on
    VectorE, broadcast across partitions with a stride-0 DMA, and the
    normalization multiply is fused into the UT psum evacuation.
  - AV: lhsT = v tiles (bf16), rhs = exp(Z^T) (bf16), nh pairs col-packed
    -> UT psum (128=(nh,a), 256=tau) -> ut fp16 (x recip).
  - Residual: lhsT = Wres halves (a, e), rhs = strided ut selection, psum
    (64=e, 512 rows); ScalarE Relu+bias -> SBUF -> contiguous DMA into a
    (64, 8192) staging output; host un-permutes to (B, S, F*E) fp32.
"""
import numpy as np

B, S, F, E, A, NH = 256, 8, 32, 64, 64, 2
NCORES = 8
BC = B // NCORES            # 32 batches per core
ROWS = BC * S               # 256 projection rows
CD = F * E                  # 2048 contraction dim
ND = A * F * NH             # 4096 projection cols
KTILES = CD // 128          # 16
TTILES = ND // 128          # 32 column tiles per weight
NB = BC * NH                # 64 attention batches per core
WCHUNK = 4                  # weight tiles per DMA
GT = 4                      # projection tiles batched per psum/copy group

_NC_CACHE = None


def build_bass():
    import concourse.bacc as bacc
    import concourse.tile as tile
    from concourse import mybir

    f16 = mybir.dt.float16
    bf16 = mybir.dt.bfloat16
    f32 = mybir.dt.float32
    Exp = mybir.ActivationFunctionType.Exp
    Relu = mybir.ActivationFunctionType.Relu

    nc = bacc.Bacc("TRN2", target_bir_lowering=False, debug=False)

    # host-prepped layouts (see make_in_maps)
    hst_d = nc.dram_tensor("hst", [128, KTILES, ROWS], f16, kind="ExternalInput")
    hsv_d = nc.dram_tensor("hsv", [128, NB // 2, 128], f16, kind="ExternalInput")
    wq_d = nc.dram_tensor("wq", [128, TTILES, KTILES * 128], f16,
                          kind="ExternalInput")
    wk_d = nc.dram_tensor("wk", [128, TTILES, KTILES * 128], f16,
                          kind="ExternalInput")
    wv_d = nc.dram_tensor("wv", [E, 2 * A], f16, kind="ExternalInput")
    wres_d = nc.dram_tensor("wres", [2 * A, E], f16, kind="ExternalInput")
    bias_d = nc.dram_tensor("bias", [E, 1], f32, kind="ExternalInput")
    out_d = nc.dram_tensor("out", [E, BC * S * F], f32, kind="ExternalOutput")

    with tile.TileContext(nc) as tc:
        from contextlib import ExitStack
        with ExitStack() as ctx:
            singles = ctx.enter_context(tc.tile_pool(name="singles", bufs=1))

            # ---- constants / persistent tiles ----
            hsT = singles.tile([128, KTILES, ROWS], f16)
            nc.gpsimd.dma_start(hsT[:, :, :], hst_d[:])
            hsv = singles.tile([128, NB // 2, 128], f16)
            nc.gpsimd.dma_start(hsv[:, :, :], hsv_d[:])

            wv_sb = singles.tile([128, 2 * A], f16)
            nc.sync.dma_start(wv_sb[0:64, :], wv_d[:])
            nc.sync.dma_start(wv_sb[64:128, :], wv_d[:])

            wres_sb = singles.tile([128, 2, E], f16)
            for half in range(2):
                for jh in range(2):
                    nc.sync.dma_start(
                        wres_sb[half * 64:(half + 1) * 64, jh, :],
                        wres_d[jh * 64:(jh + 1) * 64, :])

            bias_sb = singles.tile([E, 1], f32)
            nc.sync.dma_start(bias_sb[:, :], bias_d[:])

            qt = singles.tile([64, 2, BC, NH, 128], f16)
            kt_ = singles.tile([64, 2, BC, NH, 128], f16)
            v_all = singles.tile([128, NB, 2, A], bf16)
            ut = singles.tile([128, BC, 2, 128], f16)  # (nh,a) x (b, jh, f*4+sp)

            # ---- Q/K projection + batched gathers ----
            with tc.tile_pool(name="wtile", bufs=2) as w_pool, \
                 tc.tile_pool(name="stage", bufs=1) as st_pool, \
                 tc.tile_pool(name="pp", bufs=2, space="PSUM") as pp_pool:
                for w_d, dest, cp_eng, dma_eng in (
                        (wq_d, qt, nc.scalar, nc.sync),
                        (wk_d, kt_, nc.vector, nc.gpsimd)):
                    stage = st_pool.tile([128, BC, NH, 128], f16,
                                         name="stage", tag="stage")
                    for tg in range(TTILES // WCHUNK):
                        wt = w_pool.tile([128, WCHUNK, KTILES, 128], f16,
                                         name="wt", tag="wt")
                        dma_eng.dma_start(
                            wt[:, :, :, :],
                            w_d[:, tg * WCHUNK:(tg + 1) * WCHUNK, :]
                            .rearrange("p t (kt c) -> p t kt c", c=128))
                        for gi in range(WCHUNK // GT):
                            pp = pp_pool.tile([128, GT, ROWS], f32)
                            for ti in range(GT):
                                t = tg * WCHUNK + gi * GT + ti
                                for kt in range(KTILES):
                                    nc.tensor.matmul(
                                        pp[:, ti, :],
                                        lhsT=wt[:, gi * GT + ti, kt, :],
                                        rhs=hsT[:, kt, :],
                                        start=(kt == 0),
                                        stop=(kt == KTILES - 1))
                            # psum free (ti, b, nh, sp) -> iterate (bn, ti, sp)
                            src = pp.rearrange(
                                "p ti (b n sp) -> p (b n) ti sp", n=NH, sp=4)
                            t0 = tg * WCHUNK + gi * GT
                            dv = dest[:, 0, :, :, :].rearrange(
                                "p b n (f sp) -> p (b n) f sp", sp=4)
                            sv = stage[:, :, :, :].rearrange(
                                "p b n (f sp) -> p (b n) f sp", sp=4)
                            if cp_eng is nc.scalar:
                                cp_eng.copy(
                                    dv[:, :, t0:t0 + GT, :], src[0:64])
                                cp_eng.copy(
                                    sv[64:128, :, t0:t0 + GT, :], src[64:128])
                            else:
                                cp_eng.tensor_copy(
                                    dv[:, :, t0:t0 + GT, :], src[0:64])
                                cp_eng.tensor_copy(
                                    sv[64:128, :, t0:t0 + GT, :], src[64:128])
                    # partition shift 64..127 -> 0..63 via SBUF->SBUF DMA
                    nc.gpsimd.dma_start(
                        dest[:, 1, :, :, :],
                        stage[64:128, :, :, :])

            # ---- v projection (emitted here to fill the proj->attn gap) ----
            with tc.tile_pool(name="vps", bufs=2, space="PSUM") as vps_pool:
                for bpair in range(0, NB, 2):
                    vps = [vps_pool.tile([128, 2 * A], f32, name=f"vps{i}",
                                         tag=f"vps{i}")
                           for i in range(2)]
                    for pi in range(2):
                        nc.tensor.matmul(
                            vps[pi][:, :],
                            lhsT=hsv[pi * 64:(pi + 1) * 64, bpair // 2, :],
                            rhs=wv_sb[pi * 64:(pi + 1) * 64, :],
                            start=True, stop=True,
                            tile_position=(pi * 64, 0))
                    for pi in range(2):
                        nc.vector.tensor_copy(
                            v_all[:, bpair + pi, :, :], vps[pi][:, :])

            # ---- attention (transpose-free, Z^T layout) ----
            from concourse import bass_isa
            RAdd = bass_isa.ReduceOp.add
            with tc.tile_pool(name="zps", bufs=2, space="PSUM") as z_pool, \
                 tc.tile_pool(name="aps", bufs=2, space="PSUM") as a_pool, \
                 tc.tile_pool(name="expz", bufs=3) as e_pool, \
                 tc.tile_pool(name="dsum", bufs=2) as ds_pool, \
                 tc.tile_pool(name="reps", bufs=2) as rp_pool:
                for b in range(BC):
                    av = a_pool.tile([128, 256], f32)
                    rep = rp_pool.tile([128, 256], f32)
                    ds = ds_pool.tile([128, 256], f32, name="ds", tag="ds")
                    ezs = []
                    for nh in range(NH):
                        zt = z_pool.tile([128, 2, 256], f32, name="zt", tag="zt")
                        for h in range(2):
                            nc.tensor.matmul(
                                zt[:, h, :],
                                lhsT=kt_[:, h, b, nh, :],
                                rhs=qt[:, :, b, nh, :],
                                start=True, stop=True)
                        ez = e_pool.tile([128, 2, 256], bf16, name="ez", tag="ez")
                        ezs.append(ez)
                        nc.scalar.activation(
                            ez[:, :, :].rearrange("p a b -> p (a b)"),
                            zt[:, :, :].rearrange("p a b -> p (a b)"), Exp)
                        # denominators: all-reduce over sigma partitions
                        # (result replicated on every partition) + chunk sum
                        # on idle GpSimd; per-b reciprocal on VectorE.
                        ts_ = ds_pool.tile([128, 2, 256], f32, name="ts",
                                           tag="ts")
                        nc.gpsimd.partition_all_reduce(
                            ts_[:, :, :].rearrange("p a b -> p (a b)"),
                            ez[:, :, :].rearrange("p a b -> p (a b)"),
                            128, RAdd)
                        sl = slice(nh * 64, (nh + 1) * 64)
                        nc.gpsimd.tensor_add(
                            ds[sl, :], ts_[sl, 0, :], ts_[sl, 1, :])
                    nc.vector.reciprocal(rep[:, :], ds[:, :])
                    for nh in range(NH):
                        bn = b * NH + nh
                        for kk in range(2):
                            nc.tensor.matmul(
                                av[nh * 64:(nh + 1) * 64, :],
                                lhsT=v_all[:, bn, kk, :],
                                rhs=ezs[nh][:, kk, :],
                                start=(kk == 0), stop=(kk == 1),
                                tile_position=(0, nh * 64))
                    nc.vector.tensor_mul(
                        ut[:, b, :, :].rearrange("p a b -> p (a b)"),
                        av[:, :], rep[:, :])

            # ---- residual projection + relu + output (packed layout) ----
            with tc.tile_pool(name="rps", bufs=2, space="PSUM") as r_pool, \
                 tc.tile_pool(name="fo", bufs=3) as f_pool:
                for nh in range(NH):
                    for bg in range(BC // 4):
                        rp = r_pool.tile([64, 512], f32)
                        for jh in range(2):
                            nc.tensor.matmul(
                                rp[:, :],
                                lhsT=wres_sb[nh * 64:(nh + 1) * 64, jh, :],
                                rhs=ut[nh * 64:(nh + 1) * 64,
                                       bg * 4:(bg + 1) * 4, jh, :],
                                start=(jh == 0), stop=(jh == 1),
                                tile_position=(nh * 64, 0))
                        fo = f_pool.tile([64, 512], f32)
                        nc.scalar.activation(fo[:, :], rp[:, :], Relu,
                                             bias=bias_sb[:, :])
                        nc.sync.dma_start(
                            out_d[:, (nh * (BC // 4) + bg) * 512:
                                  (nh * (BC // 4) + bg + 1) * 512],
                            fo[:, :])
    nc.compile()
    return nc


def _get_nc():
    global _NC_CACHE
    if _NC_CACHE is None:
        _NC_CACHE = build_bass()
    return _NC_CACHE


def _prep_weight(W):
    # (CD, ND) -> (128, TTILES, KTILES*128): [p, t, kt*128+j] = W[kt*128+p, t*128+j]
    return np.ascontiguousarray(
        W.astype(np.float16).reshape(KTILES, 128, TTILES, 128)
        .transpose(1, 2, 0, 3).reshape(128, TTILES, KTILES * 128))


def make_in_maps(Hs, Wq, Wk, Wv, Wres_w, Wres_b):
    wq16 = _prep_weight(Wq)
    wk16 = _prep_weight(Wk)
    wv16 = Wv.astype(np.float16)
    wres16 = Wres_w.astype(np.float16)
    bias = Wres_b.astype(np.float32).reshape(E, 1)
    hs16 = Hs.astype(np.float16)
    maps = []
    for c in range(NCORES):
        sh = hs16[c * BC:(c + 1) * BC]                      # (BC, S, CD)
        hs2d = sh.reshape(ROWS, CD)
        hst = np.ascontiguousarray(
            hs2d.reshape(ROWS, KTILES, 128).transpose(2, 1, 0))
        # v rows in sigma' order (f*4+sp):
        # hsv[pi*64+e, q, f*4+sp] = Hs[b, nh*4+sp, f, e]; bn = 2q+pi = b*NH+nh
        arr = sh.reshape(NB, 4, F, E).transpose(0, 2, 1, 3).reshape(NB, 128, E)
        hsv = np.ascontiguousarray(
            arr.reshape(NB // 2, 2, 128, E).transpose(1, 3, 0, 2)
            .reshape(128, NB // 2, 128))
        maps.append({
            "hst": hst, "hsv": hsv,
            "wq": wq16, "wk": wk16, "wv": wv16, "wres": wres16, "bias": bias,
        })
    return maps


def _unpack_out(o):
    # o: (E, BC*S*F) = (e, nh, bg, b4, f, sp) -> (BC, S, F*E)
    o = o.reshape(E, NH, BC // 4, 4, F, 4)
    return np.ascontiguousarray(
        o.transpose(2, 3, 1, 5, 4, 0)).reshape(BC, S, F * E)


def kernel(Hs, Wq, Wk, Wv, Wres_w, Wres_b):
    from concourse.bass_utils import run_bass_kernel_spmd
    nc = _get_nc()
    in_maps = make_in_maps(Hs, Wq, Wk, Wv, Wres_w, Wres_b)
    res = run_bass_kernel_spmd(nc, in_maps, list(range(NCORES)))
    out = np.concatenate(
        [_unpack_out(np.asarray(res.results[c]["out"]))
         for c in range(NCORES)], axis=0)
    return out.astype(np.float32)


if __name__ == "__main__":
    nc = build_bass()
    print("built OK; instructions:",
          sum(len(bb.instructions) for fn in nc.m.functions
              for bb in fn.blocks))


# revision 26
# speedup vs baseline: 2.5871x; 2.5871x over previous
"""Trainium2 Bass kernel for nn_MultiHeadSelfAttention_88725434400988.

Self-contained: accepts FULL inputs, shards batch B=256 over 8 NeuronCores
(32 per core), runs one SPMD Bass program, gathers the FULL output.

Per-core algorithm (B_CORE=32, S=8, F=32, E=64, A=64, NH=2):
  - Hs, Wq, Wk, Wv, Wres cast to fp16 on host (PE matmuls run 1 cyc/row,
    fp32 PSUM accumulation; end-to-end error vs fp32 reference ~3.6e-3
    absmax / 1.9e-3 l2-relative).
  - All transposed/tiled operands are pre-laid-out on HOST into HBM buffers
    whose per-partition bytes are contiguous, so every big DMA moves
    multi-KB descriptor runs.
  - Working label order for attention rows/cols: p = jh*128 + f*4 + sp,
    where the original index is sp*64 + 2f + jh (jh == psum partition half
    of the projection output; the order makes gather copies 3-dim strided
    with 32B-contiguous runs).
  - QK projection: lhsT = 128-col tiles of W, rhs = Hs^T; psum groups of 4
    tiles (128=(jh,a), 4, 256=(b,s)); one batched copy per (group, jh).
    The jh=1 halves are staged and partition-shifted with one SBUF->SBUF
    DMA (engines cannot cross partitions; DMA can).
  - v: lhsT = host-transposed Hs rows (e, 128 rows) per (b,nh) pair ->
    v_all bf16 (128=sigma tile, bn, 2, 64).
  - Attention is TRANSPOSE-FREE: Z^T = k_chunk.T @ q (sigma on partition),
    exp on ScalarE -> bf16 (fp32 range, no overflow at |z|<=49; fp16 would
    overflow), denominators via ones-vector matmul (free), reciprocal on
    VectorE, broadcast across partitions with a stride-0 DMA, and the
    normalization multiply is fused into the UT psum evacuation.
  - AV: lhsT = v tiles (bf16), rhs = exp(Z^T) (bf16), nh pairs col-packed
    -> UT psum (128=(nh,a), 256=tau) -> ut fp16 (x recip).
  - Residual: lhsT = Wres halves (a, e), rhs = strided ut selection, psum
    (64=e, 512 rows); ScalarE Relu+bias -> SBUF -> contiguous DMA into a
    (64, 8192) staging output; host un-permutes to (B, S, F*E) fp32.
"""
import numpy as np

B, S, F, E, A, NH = 256, 8, 32, 64, 64, 2
NCORES = 8
BC = B // NCORES            # 32 batches per core
ROWS = BC * S               # 256 projection rows
CD = F * E                  # 2048 contraction dim
ND = A * F * NH             # 4096 projection cols
KTILES = CD // 128          # 16
TTILES = ND // 128          # 32 column tiles per weight
NB = BC * NH                # 64 attention batches per core
WCHUNK = 4                  # weight tiles per DMA
GT = 4                      # projection tiles batched per psum/copy group

_NC_CACHE = None


def build_bass():
    import concourse.bacc as bacc
    import concourse.tile as tile
    from concourse import mybir

    f16 = mybir.dt.float16
    bf16 = mybir.dt.bfloat16
    f32 = mybir.dt.float32
    Exp = mybir.ActivationFunctionType.Exp
    Relu = mybir.ActivationFunctionType.Relu

    nc = bacc.Bacc("TRN2", target_bir_lowering=False, debug=False)

    # host-prepped layouts (see make_in_maps)
    hst_d = nc.dram_tensor("hst", [128, KTILES, ROWS], f16, kind="ExternalInput")
    hsv_d = nc.dram_tensor("hsv", [128, NB // 2, 128], f16, kind="ExternalInput")
    wq_d = nc.dram_tensor("wq", [128, TTILES, KTILES * 128], f16,
                          kind="ExternalInput")
    wk_d = nc.dram_tensor("wk", [128, TTILES, KTILES * 128], f16,
                          kind="ExternalInput")
    wv_d = nc.dram_tensor("wv", [E, 2 * A], f16, kind="ExternalInput")
    wres_d = nc.dram_tensor("wres", [2 * A, E], f16, kind="ExternalInput")
    bias_d = nc.dram_tensor("bias", [E, 1], f32, kind="ExternalInput")
    out_d = nc.dram_tensor("out", [E, BC * S * F], f32, kind="ExternalOutput")

    with tile.TileContext(nc) as tc:
        from contextlib import ExitStack
        with ExitStack() as ctx:
            singles = ctx.enter_context(tc.tile_pool(name="singles", bufs=1))

            # ---- constants / persistent tiles ----
            ones_bf = singles.tile([128, 1], bf16)
            nc.vector.memset(ones_bf, 1.0)

            hsT = singles.tile([128, KTILES, ROWS], f16)
            nc.gpsimd.dma_start(hsT[:, :, :], hst_d[:])
            hsv = singles.tile([128, NB // 2, 128], f16)
            nc.gpsimd.dma_start(hsv[:, :, :], hsv_d[:])

            wv_sb = singles.tile([128, 2 * A], f16)
            nc.sync.dma_start(wv_sb[0:64, :], wv_d[:])
            nc.sync.dma_start(wv_sb[64:128, :], wv_d[:])

            wres_sb = singles.tile([128, 2, E], f16)
            for half in range(2):
                for jh in range(2):
                    nc.sync.dma_start(
                        wres_sb[half * 64:(half + 1) * 64, jh, :],
                        wres_d[jh * 64:(jh + 1) * 64, :])

            bias_sb = singles.tile([E, 1], f32)
            nc.sync.dma_start(bias_sb[:, :], bias_d[:])

            qt = singles.tile([64, 2, BC, NH, 128], f16)
            kt_ = singles.tile([64, 2, BC, NH, 128], f16)
            v_all = singles.tile([128, NB, 2, A], bf16)
            ut = singles.tile([128, BC, 2, 128], f16)  # (nh,a) x (b, jh, f*4+sp)

            # ---- Q/K projection + batched gathers ----
            with tc.tile_pool(name="wtile", bufs=2) as w_pool, \
                 tc.tile_pool(name="stage", bufs=1) as st_pool, \
                 tc.tile_pool(name="pp", bufs=2, space="PSUM") as pp_pool:
                for w_d, dest, cp_eng, dma_eng in (
                        (wq_d, qt, nc.scalar, nc.sync),
                        (wk_d, kt_, nc.vector, nc.gpsimd)):
                    stage = st_pool.tile([128, BC, NH, 128], f16,
                                         name="stage", tag="stage")
                    for tg in range(TTILES // WCHUNK):
                        wt = w_pool.tile([128, WCHUNK, KTILES, 128], f16,
                                         name="wt", tag="wt")
                        dma_eng.dma_start(
                            wt[:, :, :, :],
                            w_d[:, tg * WCHUNK:(tg + 1) * WCHUNK, :]
                            .rearrange("p t (kt c) -> p t kt c", c=128))
                        for gi in range(WCHUNK // GT):
                            pp = pp_pool.tile([128, GT, ROWS], f32)
                            for ti in range(GT):
                                t = tg * WCHUNK + gi * GT + ti
                                for kt in range(KTILES):
                                    nc.tensor.matmul(
                                        pp[:, ti, :],
                                        lhsT=wt[:, gi * GT + ti, kt, :],
                                        rhs=hsT[:, kt, :],
                                        start=(kt == 0),
                                        stop=(kt == KTILES - 1))
                            # psum free (ti, b, nh, sp) -> iterate (bn, ti, sp)
                            src = pp.rearrange(
                                "p ti (b n sp) -> p (b n) ti sp", n=NH, sp=4)
                            t0 = tg * WCHUNK + gi * GT
                            dv = dest[:, 0, :, :, :].rearrange(
                                "p b n (f sp) -> p (b n) f sp", sp=4)
                            sv = stage[:, :, :, :].rearrange(
                                "p b n (f sp) -> p (b n) f sp", sp=4)
                            if cp_eng is nc.scalar:
                                cp_eng.copy(
                                    dv[:, :, t0:t0 + GT, :], src[0:64])
                                cp_eng.copy(
                                    sv[64:128, :, t0:t0 + GT, :], src[64:128])
                            else:
                                cp_eng.tensor_copy(
                                    dv[:, :, t0:t0 + GT, :], src[0:64])
                                cp_eng.tensor_copy(
                                    sv[64:128, :, t0:t0 + GT, :], src[64:128])
                    # partition shift 64..127 -> 0..63 via SBUF->SBUF DMA
                    nc.gpsimd.dma_start(
                        dest[:, 1, :, :, :],
                        stage[64:128, :, :, :])

            # ---- v projection (emitted here to fill the proj->attn gap) ----
            with tc.tile_pool(name="vps", bufs=2, space="PSUM") as vps_pool:
                for bpair in range(0, NB, 2):
                    vps = [vps_pool.tile([128, 2 * A], f32, name=f"vps{i}",
                                         tag=f"vps{i}")
                           for i in range(2)]
                    for pi in range(2):
                        nc.tensor.matmul(
                            vps[pi][:, :],
                            lhsT=hsv[pi * 64:(pi + 1) * 64, bpair // 2, :],
                            rhs=wv_sb[pi * 64:(pi + 1) * 64, :],
                            start=True, stop=True,
                            tile_position=(pi * 64, 0))
                    for pi in range(2):
                        nc.vector.tensor_copy(
                            v_all[:, bpair + pi, :, :], vps[pi][:, :])

            # ---- attention (transpose-free, Z^T layout) ----
            with tc.tile_pool(name="zps", bufs=2, space="PSUM") as z_pool, \
                 tc.tile_pool(name="dps", bufs=2, space="PSUM") as d_pool, \
                 tc.tile_pool(name="aps", bufs=2, space="PSUM") as a_pool, \
                 tc.tile_pool(name="expz", bufs=3) as e_pool, \
                 tc.tile_pool(name="recs", bufs=2) as rc_pool, \
                 tc.tile_pool(name="reps", bufs=2) as rp_pool:
                for b in range(BC):
                    av = a_pool.tile([128, 256], f32)
                    denp = d_pool.tile([1, 2, 256], f32)
                    ezs = []
                    for nh in range(NH):
                        zt = z_pool.tile([128, 2, 256], f32, name="zt", tag="zt")
                        for h in range(2):
                            nc.tensor.matmul(
                                zt[:, h, :],
                                lhsT=kt_[:, h, b, nh, :],
                                rhs=qt[:, :, b, nh, :],
                                start=True, stop=True)
                        ez = e_pool.tile([128, 2, 256], bf16, name="ez", tag="ez")
                        ezs.append(ez)
                        nc.scalar.activation(
                            ez[:, :, :].rearrange("p a b -> p (a b)"),
                            zt[:, :, :].rearrange("p a b -> p (a b)"), Exp)
                        for h in range(2):
                            nc.tensor.matmul(
                                denp[:, nh, :], lhsT=ones_bf[:, :],
                                rhs=ez[:, h, :],
                                start=(h == 0), stop=(h == 1))
                    rec = rc_pool.tile([1, 2, 256], f32)
                    nc.vector.reciprocal(
                        rec[:, :, :].rearrange("p a b -> p (a b)"),
                        denp[:, :, :].rearrange("p a b -> p (a b)"))
                    # rep[0:64] = recip(nh=0), rep[64:128] = recip(nh=1):
                    # broadcasts must start at partition 0 (HW), so assemble
                    # the upper half from a full-width broadcast + p->p copy.
                    rep = rp_pool.tile([128, 256], f32)
                    repb = rp_pool.tile([128, 256], f32, name="repb", tag="repb")
                    nc.gpsimd.partition_broadcast(
                        rep[0:64, :], rec[:, 0, :], channels=64)
                    nc.gpsimd.partition_broadcast(
                        repb[:, :], rec[:, 1, :])
                    nc.gpsimd.tensor_copy(rep[64:128, :], repb[64:128, :])
                    for nh in range(NH):
                        bn = b * NH + nh
                        for kk in range(2):
                            nc.tensor.matmul(
                                av[nh * 64:(nh + 1) * 64, :],
                                lhsT=v_all[:, bn, kk, :],
                                rhs=ezs[nh][:, kk, :],
                                start=(kk == 0), stop=(kk == 1),
                                tile_position=(0, nh * 64))
                    nc.vector.tensor_mul(
                        ut[:, b, :, :].rearrange("p a b -> p (a b)"),
                        av[:, :], rep[:, :])

            # ---- residual projection + relu + output (packed layout) ----
            with tc.tile_pool(name="rps", bufs=2, space="PSUM") as r_pool, \
                 tc.tile_pool(name="fo", bufs=3) as f_pool:
                for nh in range(NH):
                    for bg in range(BC // 4):
                        rp = r_pool.tile([64, 512], f32)
                        for jh in range(2):
                            nc.tensor.matmul(
                                rp[:, :],
                                lhsT=wres_sb[nh * 64:(nh + 1) * 64, jh, :],
                                rhs=ut[nh * 64:(nh + 1) * 64,
                                       bg * 4:(bg + 1) * 4, jh, :],
                                start=(jh == 0), stop=(jh == 1),
                                tile_position=(nh * 64, 0))
                        fo = f_pool.tile([64, 512], f32)
                        nc.scalar.activation(fo[:, :], rp[:, :], Relu,
                                             bias=bias_sb[:, :])
                        nc.sync.dma_start(
                            out_d[:, (nh * (BC // 4) + bg) * 512:
                                  (nh * (BC // 4) + bg + 1) * 512],
                            fo[:, :])
    nc.compile()
    return nc


def _get_nc():
    global _NC_CACHE
    if _NC_CACHE is None:
        _NC_CACHE = build_bass()
    return _NC_CACHE


def _prep_weight(W):
    # (CD, ND) -> (128, TTILES, KTILES*128): [p, t, kt*128+j] = W[kt*128+p, t*128+j]
    return np.ascontiguousarray(
        W.astype(np.float16).reshape(KTILES, 128, TTILES, 128)
        .transpose(1, 2, 0, 3).reshape(128, TTILES, KTILES * 128))


def make_in_maps(Hs, Wq, Wk, Wv, Wres_w, Wres_b):
    wq16 = _prep_weight(Wq)
    wk16 = _prep_weight(Wk)
    wv16 = Wv.astype(np.float16)
    wres16 = Wres_w.astype(np.float16)
    bias = Wres_b.astype(np.float32).reshape(E, 1)
    hs16 = Hs.astype(np.float16)
    maps = []
    for c in range(NCORES):
        sh = hs16[c * BC:(c + 1) * BC]                      # (BC, S, CD)
        hs2d = sh.reshape(ROWS, CD)
        hst = np.ascontiguousarray(
            hs2d.reshape(ROWS, KTILES, 128).transpose(2, 1, 0))
        # v rows in sigma' order (f*4+sp):
        # hsv[pi*64+e, q, f*4+sp] = Hs[b, nh*4+sp, f, e]; bn = 2q+pi = b*NH+nh
        arr = sh.reshape(NB, 4, F, E).transpose(0, 2, 1, 3).reshape(NB, 128, E)
        hsv = np.ascontiguousarray(
            arr.reshape(NB // 2, 2, 128, E).transpose(1, 3, 0, 2)
            .reshape(128, NB // 2, 128))
        maps.append({
            "hst": hst, "hsv": hsv,
            "wq": wq16, "wk": wk16, "wv": wv16, "wres": wres16, "bias": bias,
        })
    return maps


def _unpack_out(o):
    # o: (E, BC*S*F) = (e, nh, bg, b4, f, sp) -> (BC, S, F*E)
    o = o.reshape(E, NH, BC // 4, 4, F, 4)
    return np.ascontiguousarray(
        o.transpose(2, 3, 1, 5, 4, 0)).reshape(BC, S, F * E)


def kernel(Hs, Wq, Wk, Wv, Wres_w, Wres_b):
    from concourse.bass_utils import run_bass_kernel_spmd
    nc = _get_nc()
    in_maps = make_in_maps(Hs, Wq, Wk, Wv, Wres_w, Wres_b)
    res = run_bass_kernel_spmd(nc, in_maps, list(range(NCORES)))
    out = np.concatenate(
        [_unpack_out(np.asarray(res.results[c]["out"]))
         for c in range(NCORES)], axis=0)
    return out.astype(np.float32)


if __name__ == "__main__":
    nc = build_bass()
    print("built OK; instructions:",
          sum(len(bb.instructions) for fn in nc.m.functions
              for bb in fn.blocks))
